# revision 1
# baseline (speedup 1.0000x reference)
"""Causal self-attention (nn_CausalSelfAttention) on 8 TRN2 NeuronCores.

Reference computation (B=2, T=2048, C=1024, H=16 heads, D=64):
    qkv = x @ W_attn.T + b_attn ; split q,k,v
    y   = softmax(causal(q k^T / sqrt(D))) v        (per head)
    out = y @ W_proj.T + b_proj

Sharding: batch (2-way) x head-group (4-way, 4 heads each) -> 8 cores.
Each core computes its batch's attention for its 4 heads plus the partial
c_proj contribution of those heads' channels; the host sums the 4 partials
per batch and adds the (adjusted) bias once.

Device-side simplifications (exact up to fp error):
  - k bias dropped: it shifts every score in a softmax row by the same
    constant, which cancels in softmax.
  - v bias folded into the host-side output bias: sum(P)=1 per row, so
    y = P v + bv and the bv term becomes W_p @ bv_full added once on host.

Per-core kernel, strip-pipelined over 512-query strips (s=0..3):
  - per strip: k tiles [128,128] (two heads stacked at partitions 0-63 /
    64-127), q [128,512] (+bias via DVE tensor_scalar), v tiles with a
    per-head ones column (softmax denominator accumulates in PV row 64)
  - per 128-key round: S^T per head (K=64 matmul) -> PSUM [128,512];
    ACT Exp (the only ACT work: the queue is kept exp-pure) -> bf16 P^T;
    causal diagonal blocks masked via gpsimd affine_select; PV matmul
    M=65 accumulated over rounds (one chain per PSUM bank, lagged one
    round behind S/exp so bank recycling never stalls the S stream)
  - normalize: denominator row staged to a base-0 tile (custom DVE /
    gpsimd ops only address partition 0), reciprocal_approx_fast,
    partition_broadcast, DVE multiply
  - projection emits out^T = (wp tile).T @ y strip; host transposes back

Scheduling: the PE queue is FIFO, so qkv production of strip s+1 and
projection of strip s-1 are emitted as whole-group "background units"
drip-fed between attention rounds (whole chains keep LDWEIGHTS
pipelined). PSUM: 2 banks S, 4 banks PV, 2 banks production/projection.
Known pitfalls encoded here: one accumulation chain per PSUM bank (the
chain's first matmul must cover every byte the chain touches), and
custom DVE/gpsimd ops (reciprocal_approx_fast, partition_broadcast)
read partition 0 of their input AP regardless of its base.
"""
import math
from contextlib import ExitStack

import ml_dtypes
import numpy as np

import concourse.bacc as bacc
import concourse.bass as bass
import concourse.mybir as mybir
import concourse.tile as tile
from concourse.bass_utils import run_bass_kernel_spmd

F32 = mybir.dt.float32
BF16 = mybir.dt.bfloat16
MMDT = BF16                    # dtype for all TensorE-facing tensors

N_CORES = 8
B, T, C, H = 2, 2048, 1024, 16
D = 64
GROUPS = N_CORES // B          # head groups per batch = 4
HPC = H // GROUPS              # heads per core = 4
CS = HPC * D                   # channel slice per core = 256
KT = C // 128                  # contraction tiles over C = 8
NS = T // 512                  # 512-wide query strips = 4
TT = T // 128                  # 128-row key tiles = 16


def build_nc():
    nc = bacc.Bacc("TRN2", target_bir_lowering=False, debug=False,
                   num_devices=N_CORES)

    xT = nc.dram_tensor("xT", [C, T], MMDT, kind="ExternalInput")
    wqkT = nc.dram_tensor("wqkT", [C, 2 * CS], MMDT, kind="ExternalInput")
    bq = nc.dram_tensor("bq", [2, 128, 1], F32, kind="ExternalInput")
    wvT = nc.dram_tensor("wvT", [C, CS], MMDT, kind="ExternalInput")
    wpT = nc.dram_tensor("wpT", [CS, C], MMDT, kind="ExternalInput")
    outT = nc.dram_tensor("outT", [C, T], F32, kind="ExternalOutput")

    xTr = xT.ap().rearrange("(kt p) t -> kt p t", p=128)
    wqkr = wqkT.ap().rearrange("(kt p) n -> kt p n", p=128)
    wvr = wvT.ap().rearrange("(kt p) n -> kt p n", p=128)
    wpr = wpT.ap().rearrange("(kt p) n -> kt p n", p=128)

    scale = 1.0 / math.sqrt(D)

    with tile.TileContext(nc) as tc, ExitStack() as ctx:
        pw = ctx.enter_context(tc.tile_pool(name="pw", bufs=1))
        px = ctx.enter_context(tc.tile_pool(name="px", bufs=1))
        pq = ctx.enter_context(tc.tile_pool(name="pq", bufs=1))
        pk = ctx.enter_context(tc.tile_pool(name="pk", bufs=1))
        pv = ctx.enter_context(tc.tile_pool(name="pv", bufs=1))
        py = ctx.enter_context(tc.tile_pool(name="py", bufs=1))
        ppt = ctx.enter_context(tc.tile_pool(name="ppt", bufs=12))
        pnorm = ctx.enter_context(tc.tile_pool(name="pnorm", bufs=4))
        pout = ctx.enter_context(tc.tile_pool(name="pout", bufs=4))
        # PSUM 8 banks: S singles 2 + PV(M=65) 4 + production/proj 2
        psq = ctx.enter_context(tc.tile_pool(name="psq", bufs=2, space="PSUM"))
        ppv = ctx.enter_context(tc.tile_pool(name="ppv", bufs=4, space="PSUM"))
        ppm = ctx.enter_context(tc.tile_pool(name="ppm", bufs=2, space="PSUM"))

        # ---- input DMA ----
        wqk_sb, wv_sb = [], []
        for k in range(KT):
            wt = pw.tile([128, 2 * CS], MMDT, tag=f"wqk{k}", name=f"wqk{k}")
            nc.sync.dma_start(wt[:], wqkr[k])
            wqk_sb.append(wt)
        for k in range(KT):
            vt = pw.tile([128, CS], MMDT, tag=f"wv{k}", name=f"wv{k}")
            nc.gpsimd.dma_start(vt[:], wvr[k])
            wv_sb.append(vt)
        bq_sb = []
        for m in range(2):
            bt = pw.tile([128, 1], F32, tag=f"bq{m}", name=f"bq{m}")
            nc.gpsimd.dma_start(bt[:], bq.ap()[m])
            bq_sb.append(bt)
        warm = pnorm.tile([128, 1], F32, tag="warm", name="warm")
        nc.scalar.activation(warm[:], bq_sb[0][:],
                             mybir.ActivationFunctionType.Exp, scale=0.0)
        wp_sb = []
        for k2 in range(2):
            pt_ = pw.tile([128, C], MMDT, tag=f"wp{k2}", name=f"wp{k2}")
            nc.gpsimd.dma_start(pt_[:], wpr[k2])
            wp_sb.append(pt_)
        # x: strips 0,1 as [128,512] quarters (early start), 2,3 as halves
        xq = [[None] * 2 for _ in range(KT)]   # [k][s] s in 0,1
        xh = [None] * KT                        # [k] cols 1024:2048
        for k in range(KT):
            t_ = px.tile([128, 512], MMDT, tag=f"xq{k}_0", name=f"xq{k}_0")
            nc.scalar.dma_start(t_[:], xTr[k][:, 0:512])
            xq[k][0] = t_
        for k in range(KT):
            t_ = px.tile([128, 512], MMDT, tag=f"xq{k}_1", name=f"xq{k}_1")
            nc.sync.dma_start(t_[:], xTr[k][:, 512:1024])
            xq[k][1] = t_
        for k in range(KT):
            t_ = px.tile([128, 1024], MMDT, tag=f"xh{k}", name=f"xh{k}")
            nc.scalar.dma_start(t_[:], xTr[k][:, 1024:2048])
            xh[k] = t_

        def x_strip(k, s):
            """AP of x columns [s*512, (s+1)*512) for contraction tile k."""
            if s < 2:
                return xq[k][s][:]
            return xh[k][:, (s - 2) * 512:(s - 1) * 512]

        # persistent SBUF tensors
        q_sb = [pq.tile([128, T], MMDT, tag=f"q{m}", name=f"q{m}")
                for m in range(2)]
        k_sb = [[pk.tile([128, 128], MMDT, tag=f"k{mp}_{n}", name=f"k{mp}_{n}")
                 for n in range(TT)] for mp in range(2)]
        VA = HPC * (D + 1)     # 260: per-head [v(64) | 1.0]
        v_sb = [pv.tile([128, VA], MMDT, tag=f"v{n}", name=f"v{n}")
                for n in range(TT)]
        y_sb = [py.tile([128, T], MMDT, tag=f"y{k2}", name=f"y{k2}")
                for k2 in range(2)]

        # Background units (production / projection) are emitted as lists of
        # closures so they can be drip-fed between attention rounds at MM
        # granularity: the PE queue is FIFO, so bulk work must be interleaved
        # into the emission order to fill ACT-bound gaps without delaying the
        # next round's S matmuls.
        def unit_k(s, mp, pool=None):
            ps = (pool or ppm).tile([128, 512], F32,
                                    tag="pm" if pool is None else "sq",
                                    name="ps_k")
            steps = []
            for k in range(KT):
                steps.append(lambda k=k, ps=ps: nc.tensor.matmul(
                    ps[:],
                    wqk_sb[k][:, (2 + mp) * 128:(3 + mp) * 128],
                    x_strip(k, s),
                    start=(k == 0), stop=(k == KT - 1),
                ))

            def drain(ps=ps):
                for j in range(4):
                    nc.vector.tensor_copy(k_sb[mp][4 * s + j][:],
                                          ps[:, j * 128:(j + 1) * 128])
            steps.append(drain)
            return steps

        def unit_q(s, mp, pool=None):
            ps = (pool or ppm).tile([128, 512], F32,
                                    tag="pm" if pool is None else "sq",
                                    name="ps_q")
            steps = []
            for k in range(KT):
                steps.append(lambda k=k, ps=ps: nc.tensor.matmul(
                    ps[:],
                    wqk_sb[k][:, mp * 128:(mp + 1) * 128],
                    x_strip(k, s),
                    start=(k == 0), stop=(k == KT - 1),
                ))
            steps.append(lambda ps=ps: nc.vector.tensor_scalar(
                q_sb[mp][:, s * 512:(s + 1) * 512], ps[:],
                bq_sb[mp][:], None, op0=mybir.AluOpType.add))
            return steps

        def unit_v(s, j, pool=None):
            n = 4 * s + j
            ps = (pool or ppm).tile([128, 512], F32,
                                    tag="pm" if pool is None else "sq",
                                    name="ps_v")
            steps = []
            for k in range(KT):
                steps.append(lambda k=k, ps=ps: nc.tensor.matmul(
                    ps[:, 0:CS],
                    x_strip(k, s)[:, j * 128:(j + 1) * 128],
                    wv_sb[k][:],
                    start=(k == 0), stop=(k == KT - 1),
                ))

            def drain(ps=ps):
                vgrp = v_sb[n][:].rearrange("p (g e) -> p g e", e=D + 1)
                vsrc = ps[:, 0:CS].rearrange("p (g e) -> p g e", e=D)
                nc.vector.tensor_copy(vgrp[:, :, 0:D], vsrc)
                nc.vector.memset(vgrp[:, :, D:D + 1], 1.0)
            steps.append(drain)
            return steps

        def unit_proj(s, ct, pool=None):
            ps = (pool or ppm).tile([128, 512], F32,
                                    tag="pm" if pool is None else "sq",
                                    name="ps_o")
            steps = []
            for k2 in range(2):
                steps.append(lambda k2=k2, ps=ps: nc.tensor.matmul(
                    ps[:],
                    wp_sb[k2][:, ct * 128:(ct + 1) * 128],
                    y_sb[k2][:, s * 512:(s + 1) * 512],
                    start=(k2 == 0), stop=(k2 == 1),
                ))

            def drain(ps=ps):
                ot = pout.tile([128, 512], F32, tag="ot", name="ot")
                nc.vector.tensor_copy(ot[:], ps[:])
                nc.sync.dma_start(
                    outT.ap()[ct * 128:(ct + 1) * 128,
                              s * 512:(s + 1) * 512],
                    ot[:])
            steps.append(drain)
            return steps

        def produce_units(s):
            us = []
            for mp in range(2):
                us.append(unit_k(s, mp))
            for mp in range(2):
                us.append(unit_q(s, mp))
            for j in range(4):
                us.append(unit_v(s, j))
            return us

        def run_units(units, count):
            done = 0
            while units and done < count:
                for f in units.pop(0):
                    f()
                done += 1

        # minimal upfront: k/q of strip 0 only (first S matmul needs them);
        # everything else is background work inside the strip loop
        for mp in range(2):
            for f in unit_k(0, mp, pool=psq if mp else None):
                f()
        for mp in range(2):
            for f in unit_q(0, mp, pool=psq if mp else None):
                f()
        for s in range(NS):
            bg = []
            if s == 0:
                bg += [unit_v(0, j) for j in range(4)]
            if s >= 1:
                bg += [unit_proj(s - 1, ct) for ct in range(8)]
            if s + 1 < NS:
                bg += produce_units(s + 1)
            nbg = len(bg)
            nt = 4 * s + 4
            pv_ps = [ppv.tile([D + 1, 512], F32, tag="pv", name=f"pv{h4}")
                     for h4 in range(4)]
            emitted = 0
            pv_pend = []
            for n in range(nt):
                target = (n * nbg) // nt if s == 0 else ((n + 1) * nbg) // nt
                run_units(bg, target - emitted)
                emitted = min(target, nbg)
                off = max(0, n - 4 * s) * 128
                pts = []
                for pp in range(2):
                    for r in range(2):
                        st = psq.tile([128, 512], F32, tag="sq", name="st")
                        nc.tensor.matmul(
                            st[:, off:512],
                            k_sb[pp][n][r * 64:(r + 1) * 64, :],
                            q_sb[pp][r * 64:(r + 1) * 64,
                                     s * 512 + off:(s + 1) * 512],
                            start=True, stop=True,
                        )
                        pt = ppt.tile([128, 512], MMDT, tag="pt", name="pt")
                        nc.scalar.activation(
                            pt[:, off:512], st[:, off:512],
                            mybir.ActivationFunctionType.Exp, scale=scale)
                        if n >= 4 * s:
                            nc.gpsimd.affine_select(
                                out=pt[:, off:off + 128],
                                in_=pt[:, off:off + 128],
                                compare_op=mybir.AluOpType.is_ge,
                                fill=0.0, base=0,
                                pattern=[[1, 128]], channel_multiplier=-1)
                        pts.append(pt)
                def emit_pv(n=n, off=off, pts=pts):
                    for h4 in range(4):
                        nc.tensor.matmul(
                            pv_ps[h4][:, off:512],
                            v_sb[n][:, h4 * (D + 1):(h4 + 1) * (D + 1)],
                            pts[h4][:, off:512],
                            start=(n == 0), stop=(n == nt - 1),
                        )
                pv_pend.append(emit_pv)
                if len(pv_pend) > 2:
                    pv_pend.pop(0)()

            for f in pv_pend:
                f()
            run_units(bg, 10**9)

            # ---- normalize: y = y_unnorm / denom ----
            # phase-ordered so the DVE queue never stalls behind gpsimd:
            # all copies+recips, then all broadcasts, then all multiplies.
            rrs, rbs = [], []
            for h4 in range(4):
                dtmp = pnorm.tile([1, 512], F32, tag="dtmp", name="dtmp")
                nc.vector.tensor_copy(dtmp[:], pv_ps[h4][D:D + 1, :])
                rr = pnorm.tile([1, 512], F32, tag="rr", name="rr")
                nc.vector.reciprocal_approx_fast(rr[:], dtmp[:])
                rrs.append(rr)
            for h4 in range(4):
                rb = pnorm.tile([64, 512], F32, tag="rb", name="rb")
                nc.gpsimd.partition_broadcast(rb[:], rrs[h4][:])
                rbs.append(rb)
            for pp in range(2):
                for r in range(2):
                    h4 = 2 * pp + r
                    nc.vector.tensor_tensor(
                        y_sb[pp][r * 64:(r + 1) * 64, s * 512:(s + 1) * 512],
                        pv_ps[h4][0:D, :], rbs[h4][:],
                        op=mybir.AluOpType.mult)
        for ct in range(8):
            for f in unit_proj(NS - 1, ct, pool=psq if ct % 2 else None):
                f()

    nc.compile()
    return nc


def make_in_maps(x, W_attn, b_attn, W_proj):
    """Shard full inputs into the 8 per-core input dicts."""
    x = np.asarray(x, dtype=np.float32)
    W_attn = np.asarray(W_attn, dtype=np.float32)
    b_attn = np.asarray(b_attn, dtype=np.float32)
    W_proj = np.asarray(W_proj, dtype=np.float32)
    in_maps = []
    xTb = [np.ascontiguousarray(x[b_].T) for b_ in range(B)]
    for core in range(N_CORES):
        b_ = core // GROUPS
        g = core % GROUPS
        sl = slice(g * CS, (g + 1) * CS)
        wq = W_attn[sl, :]
        wk = W_attn[C + g * CS:C + (g + 1) * CS, :]
        wv = W_attn[2 * C + g * CS:2 * C + (g + 1) * CS, :]
        bqs = b_attn[sl]
        in_maps.append({
            "xT": xTb[b_].astype(ml_dtypes.bfloat16),
            "wqkT": np.ascontiguousarray(
                np.concatenate([wq, wk], 0).T).astype(ml_dtypes.bfloat16),
            "bq": np.ascontiguousarray(bqs.reshape(2, 128, 1)),
            "wvT": np.ascontiguousarray(wv.T).astype(ml_dtypes.bfloat16),
            "wpT": np.ascontiguousarray(
                W_proj[:, g * CS:(g + 1) * CS].T).astype(ml_dtypes.bfloat16),
        })
    return in_maps


_NC = None


def _get_nc():
    global _NC
    if _NC is None:
        _NC = build_nc()
    return _NC


def run(x, W_attn, b_attn, W_proj, b_proj, trace=False):
    nc = _get_nc()
    in_maps = make_in_maps(x, W_attn, b_attn, W_proj)
    res = run_bass_kernel_spmd(nc, in_maps, core_ids=list(range(N_CORES)),
                               trace=trace)
    out = np.zeros((B, T, C), dtype=np.float32)
    for core in range(N_CORES):
        out[core // GROUPS] += res.results[core]["outT"].T
    # b_proj plus the folded-in v bias: y = P v + bv, sum(P)=1 per row
    b_eff = (np.asarray(b_proj, dtype=np.float32)
             + np.asarray(W_proj, dtype=np.float32)
             @ np.asarray(b_attn, dtype=np.float32)[2 * C:3 * C])
    out += b_eff[None, None, :]
    return out, res


def kernel(x, W_attn, b_attn, W_proj, b_proj):
    out, _ = run(x, W_attn, b_attn, W_proj, b_proj, trace=False)
    return out

